# revision 3
# baseline (speedup 1.0000x reference)
"""GCN encoder kernel for 8 Trainium2 NeuronCores.

Strategy
--------
out = relu(relu(A_hat @ x @ W0) @ W1), A_hat = D^-1/2 (A + I) D^-1/2.

- Destinations (output rows) are sharded across the 8 cores; each core owns
  N/8 nodes and all edges pointing at them.
- Host-side prep (index work only): per core, edges are bucketed by
  destination, destinations are degree-sorted into tiles of 128, and each
  edge becomes a "slot" (partition = destination's position in its tile,
  column = edge rank).  Slots are gathered from HBM with dma_gather using
  node-PAIR rows (512 B) so the int16 index (= src//2) covers all 50k nodes;
  a per-slot norm pair masks the wanted half (and merges two edges whose
  sources share a pair).  Per-edge norm = dinv[src]*dinv[dst] rides in that
  mask, so the device computes the full normalized aggregation.
- On device: dma_gather over 4 SWDGE queues (the gather is the bottleneck;
  multiple queues overlap ring drain), DVE applies the norm mask and folds
  the pair halves, TensorE accumulates slot columns into PSUM via an
  identity stationary (segment-sum), then the two dense layers run
  feature-major with fused ReLU eviction on ScalarE.
"""

import os
import sys

for _p in ("/opt/trn_rl_repo", "/root/.axon_site/_ro/trn_rl_repo"):
    if os.path.isdir(_p) and _p not in sys.path:
        sys.path.insert(0, _p)

import numpy as np
import ml_dtypes
from contextlib import ExitStack

import concourse.bass as bass
import concourse.tile as tile
from concourse import bacc, mybir
from concourse.bass_utils import run_bass_kernel_spmd
from concourse.masks import make_identity
from concourse.ap import AP

P = 128
NCORES = 8
CALL = 2048            # gather slots per dma_gather call (16 columns)
CALL_COLS = CALL // P  # 16
NQ = 4                 # SWDGE queues
bf16 = mybir.dt.bfloat16
f32 = mybir.dt.float32
i16 = mybir.dt.int16
BF = ml_dtypes.bfloat16


def _ap3(t_ap, d1, d2):
    return AP(t_ap.tensor, t_ap.offset, [t_ap.ap[0], [d2 * (t_ap.ap[-1][0]), d1], [t_ap.ap[-1][0], d2]])


def _prep(x, W0, W1, edge_index):
    N, F = x.shape
    H = W0.shape[1]
    E = edge_index.shape[1]
    ND = (N + NCORES - 1) // NCORES          # dsts per core
    NT = (ND + P - 1) // P                   # dst tiles per core
    NDP = NT * P                             # padded dsts per core

    row = np.asarray(edge_index[0], dtype=np.int64)
    col = np.asarray(edge_index[1], dtype=np.int64)
    deg = np.bincount(col, minlength=N).astype(np.float32) + 1.0
    dinv = (1.0 / np.sqrt(deg)).astype(np.float32)

    rows_all = np.concatenate([row, np.arange(N, dtype=np.int64)])
    cols_all = np.concatenate([col, np.arange(N, dtype=np.int64)])
    norm_all = dinv[rows_all] * dinv[cols_all]
    core_of = cols_all // ND

    npair = (N + 1) // 2 + 1                 # +1 zero pair
    zero_pair = npair - 1
    assert zero_pair <= 32767

    # gather source: x rows as pairs, padded with zeros
    xp = np.zeros((2 * npair, F), dtype=BF)
    xp[:N] = x.astype(BF)
    ypair = xp.reshape(npair, 2 * F)

    per_core = []
    sdeg_tiles = np.zeros((NCORES, NT), dtype=np.int64)
    for c in range(NCORES):
        m = core_of == c
        r = rows_all[m]
        dl = cols_all[m] - c * ND
        nm = norm_all[m]
        pr = r >> 1
        hf = (r & 1).astype(np.int64)
        key = dl * npair + pr
        uniq, inv = np.unique(key, return_inverse=True)
        S0 = uniq.shape[0]
        norm2 = np.zeros((S0, 2), dtype=np.float32)
        np.add.at(norm2, (inv, hf), nm)
        slot_dl = (uniq // npair).astype(np.int64)
        slot_pr = (uniq % npair).astype(np.int64)
        sdeg = np.bincount(slot_dl, minlength=NDP)
        start_of = np.zeros(NDP + 1, dtype=np.int64)
        np.cumsum(sdeg, out=start_of[1:])
        j_rank = np.arange(S0, dtype=np.int64) - start_of[slot_dl]
        perm = np.argsort(-sdeg, kind="stable")       # position -> dst
        pos_of = np.empty(NDP, dtype=np.int64)
        pos_of[perm] = np.arange(NDP)
        sdeg_sorted = sdeg[perm]
        sdeg_tiles[c] = sdeg_sorted.reshape(NT, P).max(axis=1)
        per_core.append(dict(slot_dl=slot_dl, slot_pr=slot_pr, j_rank=j_rank,
                             norm2=norm2, pos_of=pos_of))

    cols_t = np.maximum(((sdeg_tiles.max(axis=0) + 3) // 4) * 4, 4).astype(np.int64)
    colbase = np.zeros(NT + 1, dtype=np.int64)
    np.cumsum(cols_t, out=colbase[1:])
    C = int(colbase[-1])
    ncalls = (C + CALL_COLS - 1) // CALL_COLS
    C_pad = ncalls * CALL_COLS

    tile_of_col = np.full(C_pad, -1, dtype=np.int64)
    for t in range(NT):
        tile_of_col[colbase[t]:colbase[t + 1]] = t

    in_maps = []
    unshard = []
    for c in range(NCORES):
        pc = per_core[c]
        pos = pc["pos_of"][pc["slot_dl"]]
        prow = pos % P
        scol = colbase[pos // P] + pc["j_rank"]
        idx_arr = np.full((P, C_pad), zero_pair, dtype=np.int16)
        idx_arr[prow, scol] = pc["slot_pr"].astype(np.int16)
        norm2_arr = np.zeros((P, 2 * C_pad), dtype=BF)
        norm2_arr[prow, 2 * scol] = pc["norm2"][:, 0].astype(BF)
        norm2_arr[prow, 2 * scol + 1] = pc["norm2"][:, 1].astype(BF)
        # re-layout idx for dma_gather: slot i of call k lives at [i%16, k*128 + i//16]
        b2 = idx_arr.reshape(P, ncalls, CALL_COLS).transpose(1, 2, 0).reshape(ncalls, P, 16)
        idx16 = np.tile(b2.transpose(0, 2, 1), (1, NCORES, 1)).transpose(1, 0, 2).reshape(P, ncalls * P)
        in_maps.append({
            "ypair": ypair,
            "idx": np.ascontiguousarray(idx16),
            "norm2": np.ascontiguousarray(norm2_arr),
            "w0": W0.astype(BF),
            "w1lo": W1[:128].astype(BF),
            "w1hi": W1[128:].astype(BF),
        })
        unshard.append(pc["pos_of"])

    meta = dict(N=N, F=F, H=H, ND=ND, NT=NT, NDP=NDP, npair=npair,
                ncalls=ncalls, C_pad=C_pad, cols_t=cols_t.tolist(),
                colbase=colbase.tolist(), tile_of_col=tile_of_col)
    return in_maps, unshard, meta


def _build(meta):
    F, H = meta["F"], meta["H"]
    NT, NDP, npair = meta["NT"], meta["NDP"], meta["npair"]
    ncalls, C_pad = meta["ncalls"], meta["C_pad"]
    cols_t, colbase = meta["cols_t"], meta["colbase"]
    tile_of_col = meta["tile_of_col"]
    F2 = 2 * F

    nc = bacc.Bacc(None, target_bir_lowering=False, debug=False,
                   num_devices=NCORES, num_swdge_queues=NQ,
                   dynamic_dma_scratch_size=NQ * CALL * 16)
    ypair_d = nc.declare_dram_parameter("ypair", [npair, F2], bf16, isOutput=False)
    idx_d = nc.declare_dram_parameter("idx", [P, ncalls * P], i16, isOutput=False)
    norm2_d = nc.declare_dram_parameter("norm2", [P, 2 * C_pad], bf16, isOutput=False)
    w0_d = nc.declare_dram_parameter("w0", [F, H], bf16, isOutput=False)
    w1lo_d = nc.declare_dram_parameter("w1lo", [128, H], bf16, isOutput=False)
    w1hi_d = nc.declare_dram_parameter("w1hi", [H - 128, H], bf16, isOutput=False)
    out_d = nc.declare_dram_parameter("out", [H, NDP], f32, isOutput=True)

    # phase-2 chunks: groups of 4 dst tiles (512 dsts)
    chunks = [(j * 4, min(4, NT - j * 4)) for j in range((NT + 3) // 4)]

    with tile.TileContext(nc) as tc, ExitStack() as ctx:
        cpool = ctx.enter_context(tc.tile_pool(name="const", bufs=1))
        gpool = ctx.enter_context(tc.tile_pool(name="g", bufs=3))
        spool = ctx.enter_context(tc.tile_pool(name="gs", bufs=2))
        s2pool = ctx.enter_context(tc.tile_pool(name="gs2", bufs=3))
        hpool = ctx.enter_context(tc.tile_pool(name="h0", bufs=2))
        h0Tp = ctx.enter_context(tc.tile_pool(name="h0T", bufs=3))
        h1p = ctx.enter_context(tc.tile_pool(name="h1", bufs=2))
        opool = ctx.enter_context(tc.tile_pool(name="o", bufs=2))
        ps_acc = ctx.enter_context(tc.tile_pool(name="ps_acc", bufs=2, space="PSUM"))
        ps_tr = ctx.enter_context(tc.tile_pool(name="ps_tr", bufs=2, space="PSUM"))
        ps_u = ctx.enter_context(tc.tile_pool(name="ps_u", bufs=1, space="PSUM"))
        ps_v = ctx.enter_context(tc.tile_pool(name="ps_v", bufs=1, space="PSUM"))

        ident = cpool.tile([P, P], bf16)
        make_identity(nc, ident[:])
        idx_sb = cpool.tile([P, ncalls * P], i16)
        nc.sync.dma_start(idx_sb[:], idx_d[:])
        norm2_sb = cpool.tile([P, 2 * C_pad], bf16)
        nc.sync.dma_start(norm2_sb[:], norm2_d[:])
        w0_sb = cpool.tile([F, H], bf16)
        nc.sync.dma_start(w0_sb[:], w0_d[:])
        w1lo_sb = cpool.tile([128, H], bf16)
        nc.sync.dma_start(w1lo_sb[:], w1lo_d[:])
        w1hi_sb = cpool.tile([H - 128, H], bf16)
        nc.sync.dma_start(w1hi_sb[:], w1hi_d[:])

        acc_of_tile = {}
        h0T_chunk = {}

        def finish_tile(t):
            """reduce psum quarters -> transpose -> stash into h0T chunk."""
            accp, width = acc_of_tile.pop(t)
            nquad = min(4, width)
            h0tmp = hpool.tile([P, P], bf16, tag="h0tmp")
            in_ap = AP(accp[:].tensor, accp[:].offset,
                       [accp[:].ap[0], [1, P], [P, nquad]])
            with nc.allow_low_precision("bf16 h0 evict"):
                nc.vector.tensor_reduce(h0tmp[:], in_ap, axis=mybir.AxisListType.X,
                                        op=mybir.AluOpType.add, opt_input=False)
            trp = ps_tr.tile([P, P], bf16, tag="tr")
            nc.tensor.transpose(trp[:], h0tmp[:], ident[:])
            j = t // 4
            if j not in h0T_chunk:
                w = chunks[j][1] * P
                h0T_new = h0Tp.tile([P, w], bf16, tag="h0T")
                h0T_chunk[j] = h0T_new
            nc.scalar.copy(h0T_chunk[j][:, (t % 4) * P:(t % 4 + 1) * P], trp[:])
            if t % 4 == 3 or t == NT - 1:
                phase2(j)

        def phase2(j):
            t0, ntile = chunks[j]
            w = ntile * P
            h0T = h0T_chunk.pop(j)
            u1 = ps_u.tile([P, w], f32, tag="u1")
            u2 = ps_u.tile([P, w], f32, tag="u2")
            nc.tensor.matmul(u1[:], lhsT=w0_sb[:, 0:128], rhs=h0T[:], start=True, stop=True)
            nc.tensor.matmul(u2[:], lhsT=w0_sb[:, 128:H], rhs=h0T[:], start=True, stop=True)
            h1a = h1p.tile([P, w], bf16, tag="h1a")
            h1b = h1p.tile([P, w], bf16, tag="h1b")
            nc.scalar.activation(h1a[:], u1[:], mybir.ActivationFunctionType.Relu)
            nc.scalar.activation(h1b[:], u2[:], mybir.ActivationFunctionType.Relu)
            v1 = ps_v.tile([P, w], f32, tag="v1")
            v2 = ps_v.tile([P, w], f32, tag="v2")
            nc.tensor.matmul(v1[:], lhsT=w1lo_sb[:, 0:128], rhs=h1a[:], start=True, stop=False)
            nc.tensor.matmul(v1[:], lhsT=w1hi_sb[:, 0:128], rhs=h1b[:], start=False, stop=True)
            nc.tensor.matmul(v2[:], lhsT=w1lo_sb[:, 128:H], rhs=h1a[:], start=True, stop=False)
            nc.tensor.matmul(v2[:], lhsT=w1hi_sb[:, 128:H], rhs=h1b[:], start=False, stop=True)
            o1 = opool.tile([P, w], f32, tag="o1")
            o2 = opool.tile([P, w], f32, tag="o2")
            nc.scalar.activation(o1[:], v1[:], mybir.ActivationFunctionType.Relu)
            nc.scalar.activation(o2[:], v2[:], mybir.ActivationFunctionType.Relu)
            nc.sync.dma_start(out_d[0:128, t0 * P:t0 * P + w], o1[:])
            nc.sync.dma_start(out_d[128:H, t0 * P:t0 * P + w], o2[:])

        for k in range(ncalls):
            g = gpool.tile([P, CALL_COLS * F2], bf16, tag="g")
            nc.gpsimd.dma_gather(
                out_ap=_ap3(g[:], CALL_COLS, F2),
                in_ap=ypair_d[:],
                idxs_ap=idx_sb[:, k * P:(k + 1) * P],
                num_idxs=CALL, num_idxs_reg=CALL, elem_size=F2,
                single_packet=False, queue_num=k % NQ)
            gs = spool.tile([P, CALL_COLS * F2], bf16, tag="gs")
            nc.vector.tensor_tensor(
                out=gs[:], in0=g[:],
                in1=norm2_sb[:, k * 2 * CALL_COLS:(k + 1) * 2 * CALL_COLS]
                    .to_broadcast([P, 2 * CALL_COLS, F]),
                op=mybir.AluOpType.mult)
            gs2 = s2pool.tile([P, CALL_COLS * F], bf16, tag="gs2")
            ga = gs[:]
            half0 = AP(ga.tensor, ga.offset, [ga.ap[0], [F2, CALL_COLS], [1, F]])
            half1 = AP(ga.tensor, ga.offset + F, [ga.ap[0], [F2, CALL_COLS], [1, F]])
            nc.vector.tensor_tensor(out=gs2[:], in0=half0, in1=half1,
                                    op=mybir.AluOpType.add)
            for gi in range(CALL_COLS // 4):
                c0 = k * CALL_COLS + 4 * gi
                t = int(tile_of_col[c0])
                if t < 0:
                    continue
                if t not in acc_of_tile:
                    accnew = ps_acc.tile([P, 4 * F], f32, tag="acc")
                    acc_of_tile[t] = (accnew, cols_t[t])
                accp, _ = acc_of_tile[t]
                first = c0 == colbase[t]
                last = c0 + 4 >= colbase[t + 1]
                nc.tensor.matmul(accp[:], lhsT=ident[:],
                                 rhs=gs2[:, 4 * gi * F:(4 * gi + 4) * F],
                                 start=first, stop=last)
                if last:
                    finish_tile(t)
    nc.compile()
    return nc


def _run(inputs, trace=False):
    x = np.asarray(inputs["x"])
    W0 = np.asarray(inputs["W0"])
    W1 = np.asarray(inputs["W1"])
    edge_index = np.asarray(inputs["edge_index"])
    in_maps, unshard, meta = _prep(x, W0, W1, edge_index)
    nc = _build(meta)
    res = run_bass_kernel_spmd(nc, in_maps, core_ids=list(range(NCORES)), trace=trace)
    N, H, ND = meta["N"], meta["H"], meta["ND"]
    h = np.empty((N, H), dtype=np.float32)
    for c in range(NCORES):
        o = res.results[c]["out"]            # [H, NDP]
        nd_c = min(ND, N - c * ND)
        h[c * ND:c * ND + nd_c] = o.T[unshard[c][:nd_c]]
    return h, res


def kernel(**inputs) -> np.ndarray:
    h, _ = _run(inputs, trace=False)
    return h
